# revision 1
# baseline (speedup 1.0000x reference)
"""Trainium2 Bass/Tile kernel for EnrichedGeometricEmbedding.

Full-input contract: kernel(**inputs) takes the complete tensors, shards the
batch dim across 8 NeuronCores (B=8 -> 1 batch row per core), runs one SPMD
program via run_bass_kernel_spmd, and gathers the full [8, 1024, 32, 384]
output. Memory-bound problem: the 50 MB/core output write (~140 us at
~360 GB/s) sets the roofline; cost-model end-to-end is ~183 us/core.

Per-core pipeline (S=1024 groups, K=32 points/group, P=32768 points).
Points are processed internally in k-major order p' = k*1024 + g so every
DMA moves >=512-byte contiguous runs (4-byte-strided DMAs explode the
descriptor count); the output DMA scatters rows back to natural order at
1536 B/row granularity.

  1. Stats (group-major [128, t*96] tiles, two big input DMAs): neighbor
     mean + centering in single batched DVE ops; unnormalized 3x3 covariance
     via one mul+reduce pair per unique entry.
  2. Smallest eigenvalue per group, batched [128, 8], with the closed-form
     trigonometric formula. ACT-domain-safe: acos(r) = 2*arctan(sqrt((1-|r|)/
     (1+|r|))) with a sign fold, cos via sin, all args within the scalar
     engine's table domains. curv = lam_min / (trace + (K-1)*1e-6).
  3. PE transposes put x into a (d,k)-partition layout xdkT [96, 1024]
     (d-planes at partition bases 0/32/64); a PE "broadcast" matmul
     (rmat @ means) replicates group means across k-partitions for the lap
     rows. fusedT low rows [g42 rbf, curv, lap x3, ones] stream into a
     quarter-P double-buffered flo tile per phase; the ones row folds the
     projection bias into the matmul.
  4. Main loop (4 phases x 8 k x 2 halves of 512 points): xb[128,512] =
     Ebig_k^T @ xdkT broadcasts x to the 128 rbf rows (float32r matmuls run
     at 1 cyc/row for free dim >= 256); rbf rows = ACT Square(x + bias=-c)
     then Exp(scale=-2) with per-partition constants; per 128-point tile one
     K=128 (W_hi) + one K=6 (W_lo) accumulating float32r matmul into PSUM;
     PSUM->SBUF copies alternate DVE/ACT per pair; one HWDGE DMA per 512
     points scatters [128, 4, 384] to DRAM rows g*32+k.

All matmul operands are float32r end-to-end (walrus requires fp32r inputs to
be produced as fp32r); accumulation stays fp32, giving ~3e-4 relative error.
"""

import math

import numpy as np

B, S, K, D = 8, 1024, 32, 3
F = 43                      # FEAT_DIM
OUT = 384
G = S                       # groups per core
P = S * K                   # points per core (32768)
NT = G // 128               # group tiles (8)
TOTAL = F * D + 1 + D       # 133

_prog_cache = {}


def _build_program():
    import concourse.bacc as bacc
    import concourse.mybir as mybir
    from concourse.tile import TileContext, add_dep_helper

    DT = mybir.dt.float32
    DTR = mybir.dt.float32r
    Act = mybir.ActivationFunctionType
    Op = mybir.AluOpType
    X = mybir.AxisListType.X

    C = np.linspace(-1.0, 1.0, F + 2, dtype=np.float64)[1:-1]
    C42 = float(C[F - 1])

    nc = bacc.Bacc("TRN2", target_bir_lowering=False, debug=False, num_devices=8)
    xyz_d = nc.dram_tensor("xyz", [P, D], DT, kind="ExternalInput").ap()
    nbr_d = nc.dram_tensor("nbr", [P, D], DT, kind="ExternalInput").ap()
    # packed constants: blob1 = ident | wlo | negc | rmat (small, loads first);
    # blob2 = whi | ebig (big, overlaps the stats phase)
    NB1 = 128 + OUT + 1 + 96
    NB2 = OUT + K * 128
    blob1_d = nc.dram_tensor("blob1", [128, NB1], DTR, kind="ExternalInput").ap()
    blob2_d = nc.dram_tensor("blob2", [128, NB2], DTR, kind="ExternalInput").ap()
    out_d = nc.dram_tensor("out", [P, OUT], DT, kind="ExternalOutput").ap()

    def view_ti(t24, width, i):
        # column i of a [128, nt*width] tile laid out (t, i) -> [128, nt], step width
        return t24.rearrange("p (t i) -> p i t", i=width)[:, i : i + 1, :].squeeze(1)

    with TileContext(nc) as tc:
        with (
            tc.tile_pool(name="const", bufs=1) as constp,
            tc.tile_pool(name="stats", bufs=1) as statp,
            tc.tile_pool(name="gwork", bufs=8) as gwp,
            tc.tile_pool(name="flopool", bufs=1) as flop,
            tc.tile_pool(name="main", bufs=6) as mainp,
        ):
            ppsum = tc.alloc_tile_pool(name="ppsum", bufs=7, space="PSUM")
            # ---- constants ----
            nbr_g = nbr_d.rearrange("(g k) d -> g (k d)", k=K)
            xyz_g = xyz_d.rearrange("(g k) d -> g (k d)", k=K)
            n_all = gwp.tile([128, NT * K * D], DT, tag="nall", bufs=1)
            nc.scalar.dma_start(
                n_all.rearrange("p (t f) -> p t f", f=K * D),
                nbr_g.rearrange("(t p) f -> p t f", p=128),
            )
            blob1 = constp.tile([128, NB1], DTR)
            nc.scalar.dma_start(blob1[:], blob1_d[:])
            ident = blob1[:, 0:128].bitcast(DT)
            wlo = blob1[0:6, 128 : 128 + OUT]
            negc = blob1[:, 128 + OUT : 128 + OUT + 1].bitcast(DT)
            rmat = blob1[0:D, 128 + OUT + 1 : 128 + OUT + 1 + 96]
            c242_t = constp.tile([96, 1], DT)
            nc.vector.memset(c242_t[:], -2.0 * C42)
            ce42_t = constp.tile([96, 1], DT)
            nc.vector.memset(ce42_t[:], -2.0 * C42 * C42)
            bias_tiny = constp.tile([128, 1], DT)
            nc.vector.memset(bias_tiny[:], 1e-12)


            # x in (d,k)-partition layout: row 32*d + k, col g (f32r: matmul rhs)
            xdkT = statp.tile([96, G], DTR)

            # ---- stats phase: transpose x, neighbor mean, covariance ----
            U_all = statp.tile([128, NT * 6], DT)
            m_all = statp.tile([128, NT * 3], DT)
            x_all = gwp.tile([128, NT * K * D], DT, tag="xall", bufs=1)
            nc.scalar.dma_start(
                x_all.rearrange("p (t f) -> p t f", f=K * D),
                xyz_g.rearrange("(t p) f -> p t f", p=128),
            )
            blob2 = constp.tile([128, NB2], DTR)
            nc.scalar.dma_start(blob2[:], blob2_d[:])
            whi = blob2[:, 0:OUT]
            ebig = blob2[0:96, OUT : OUT + K * 128]
            for t in range(NT):
                x_kd = x_all[:, t * K * D : (t + 1) * K * D].rearrange(
                    "g (k d) -> g d k", d=D
                )
                for d in range(D):
                    xps = ppsum.tile([32, 128], DT, tag="pp")
                    nc.tensor.transpose(
                        xps[:], x_kd[:, d : d + 1, :].squeeze(1), ident
                    )
                    nc.scalar.copy(
                        xdkT[d * K : (d + 1) * K, t * 128 : (t + 1) * 128], xps[:]
                    )
            # neighbor means (all tiles at once) and centering
            n_v = n_all.rearrange("p (t k d) -> p t d k", k=K, d=D)
            m_v = m_all.rearrange("p (t d) -> p t d", d=D)
            nc.vector.tensor_reduce(m_v, n_v, axis=X, op=Op.add)
            nc.vector.tensor_scalar_mul(m_all[:], m_all[:], 1.0 / K)
            nc.vector.tensor_sub(
                n_v, n_v, m_v.unsqueeze(3).broadcast_to([128, NT, D, K])
            )
            n_tdk = n_all.rearrange("p (t k d) -> p t d k", k=K, d=D)
            U_v = U_all.rearrange("p (t i) -> p i t", i=6)
            for idx, (i, j) in enumerate(
                [(0, 0), (1, 1), (2, 2), (0, 1), (1, 2), (0, 2)]
            ):
                prod = gwp.tile([128, NT * K], DT, tag="prod", bufs=2)
                prod_v = prod.rearrange("p (t k) -> p t k", k=K)
                nc.vector.tensor_mul(
                    prod_v,
                    n_tdk[:, :, i : i + 1, :].squeeze(2),
                    n_tdk[:, :, j : j + 1, :].squeeze(2),
                )
                nc.vector.tensor_reduce(
                    U_v[:, idx : idx + 1, :].squeeze(1), prod_v, axis=X, op=Op.add
                )

            m_td = m_all.rearrange("p (t d) -> p d t", d=D)
            mdt = statp.tile([96, 128], DT)  # d-plane at partition 32*d, rows 0..7
            for d in range(D):
                mps = ppsum.tile([8, 128], DT, tag="pp")
                nc.tensor.transpose(mps[:], m_td[:, d : d + 1, :].squeeze(1), ident)
                nc.vector.tensor_copy(mdt[d * K : d * K + NT, :], mps[:])
            m3 = statp.tile([D, G], DTR)
            for d in range(D):
                nc.scalar.dma_start(
                    m3[d : d + 1, :].rearrange("o (t g) -> o t g", g=128),
                    mdt[d * K : d * K + NT, :].bitcast(DTR),
                )

            # ---- eigen phase (all 1024 groups batched [128, 8]) ----
            Ud = U_all.rearrange("p (t i) -> p t i", i=6)[:, :, 0:3]
            Uo = U_all.rearrange("p (t i) -> p t i", i=6)[:, :, 3:6]
            tr_t = statp.tile([128, NT], DT)
            nc.vector.tensor_reduce(tr_t[:], Ud, axis=X, op=Op.add)
            q_t = statp.tile([128, NT], DT)
            nc.vector.tensor_scalar_mul(q_t[:], tr_t[:], 1.0 / 3)
            sq_t = statp.tile([128, NT * 3], DT)
            sq3 = sq_t.rearrange("p (t i) -> p t i", i=3)
            nc.vector.tensor_mul(sq3, Uo, Uo)
            p1_t = statp.tile([128, NT], DT)
            nc.vector.tensor_reduce(p1_t[:], sq3, axis=X, op=Op.add)
            dd_t = statp.tile([128, NT * 3], DT)
            dd3 = dd_t.rearrange("p (t i) -> p t i", i=3)
            nc.vector.tensor_sub(dd3, Ud, q_t.unsqueeze(2).broadcast_to([128, NT, 3]))
            dd2_t = statp.tile([128, NT * 3], DT)
            dd23 = dd2_t.rearrange("p (t i) -> p t i", i=3)
            nc.vector.tensor_mul(dd23, dd3, dd3)
            s2_t = statp.tile([128, NT], DT)
            nc.vector.tensor_reduce(s2_t[:], dd23, axis=X, op=Op.add)
            p2_t = statp.tile([128, NT], DT)
            nc.vector.scalar_tensor_tensor(
                p2_t[:], p1_t[:], 2.0, s2_t[:], op0=Op.mult, op1=Op.add
            )
            p_t = statp.tile([128, NT], DT)
            nc.scalar.activation(
                p_t[:], p2_t[:], Act.Sqrt, bias=bias_tiny[:], scale=1.0 / 6
            )

            a_v = view_ti(dd_t, 3, 0)
            b_v = view_ti(dd_t, 3, 1)
            c_v = view_ti(dd_t, 3, 2)
            ff_v = view_ti(sq_t, 3, 0)
            gg_v = view_ti(sq_t, 3, 1)
            hh_v = view_ti(sq_t, 3, 2)
            f_v = view_ti(U_all, 6, 3)
            g_v = view_ti(U_all, 6, 4)
            h_v = view_ti(U_all, 6, 5)

            det = statp.tile([128, NT], DT)
            scr = statp.tile([128, NT], DT)
            nc.vector.tensor_mul(det[:], a_v, b_v)
            nc.vector.tensor_mul(det[:], det[:], c_v)          # abc
            nc.vector.tensor_mul(scr[:], f_v, g_v)
            nc.vector.scalar_tensor_tensor(
                scr[:], scr[:], 2.0, h_v, op0=Op.mult, op1=Op.mult
            )                                                  # 2fgh
            nc.vector.tensor_add(det[:], det[:], scr[:])
            nc.vector.tensor_mul(scr[:], a_v, gg_v)
            nc.vector.tensor_sub(det[:], det[:], scr[:])
            nc.vector.tensor_mul(scr[:], b_v, hh_v)
            nc.vector.tensor_sub(det[:], det[:], scr[:])
            nc.vector.tensor_mul(scr[:], c_v, ff_v)
            nc.vector.tensor_sub(det[:], det[:], scr[:])

            pp_t = statp.tile([128, NT], DT)
            nc.vector.tensor_mul(pp_t[:], p_t[:], p_t[:])
            nc.vector.tensor_mul(pp_t[:], pp_t[:], p_t[:])
            nc.vector.tensor_scalar_mul(pp_t[:], pp_t[:], 2.0)  # 2 p^3
            rec_t = statp.tile([128, NT], DT)
            nc.vector.reciprocal(rec_t[:], pp_t[:])
            r_t = statp.tile([128, NT], DT)
            nc.vector.tensor_mul(r_t[:], det[:], rec_t[:])
            nc.vector.tensor_scalar(
                r_t[:], r_t[:], 0.999999, -0.999999, op0=Op.min, op1=Op.max
            )
            # acos(r) = 2*arctan(sqrt((1-|r|)/(1+|r|))) with sign fold-in;
            # lam_min = q - 2p*sin(acos(r)/3 + pi/6), sin arg in [pi/6, pi/2]
            u_t = statp.tile([128, NT], DT)
            nc.scalar.activation(u_t[:], r_t[:], Act.Abs)
            num_t = statp.tile([128, NT], DT)
            nc.vector.tensor_scalar(
                num_t[:], u_t[:], -1.0, 1.0, op0=Op.mult, op1=Op.add
            )  # 1 - u
            den2_t = statp.tile([128, NT], DT)
            nc.vector.tensor_scalar_add(den2_t[:], u_t[:], 1.0)  # 1 + u
            dr2_t = statp.tile([128, NT], DT)
            nc.vector.reciprocal(dr2_t[:], den2_t[:])
            w_t = statp.tile([128, NT], DT)
            nc.vector.tensor_mul(w_t[:], num_t[:], dr2_t[:])
            v_t = statp.tile([128, NT], DT)
            nc.scalar.activation(v_t[:], w_t[:], Act.Sqrt)
            at_t = statp.tile([128, NT], DT)
            nc.scalar.activation(at_t[:], v_t[:], Act.Arctan)
            sgn_t = statp.tile([128, NT], DT)
            nc.scalar.activation(sgn_t[:], r_t[:], Act.Sign)
            z_t = statp.tile([128, NT], DT)
            nc.vector.tensor_scalar(
                z_t[:], at_t[:], 2.0 / 3.0, -math.pi / 6.0, op0=Op.mult, op1=Op.add
            )
            arg_t = statp.tile([128, NT], DT)
            nc.vector.scalar_tensor_tensor(
                arg_t[:], sgn_t[:], 1.0, z_t[:], op0=Op.mult, op1=Op.mult
            )
            nc.vector.tensor_scalar_add(arg_t[:], arg_t[:], math.pi / 3.0)
            sv_t = statp.tile([128, NT], DT)
            sin_inst = nc.scalar.activation(sv_t[:], arg_t[:], Act.Sin)
            lam_t = statp.tile([128, NT], DT)
            nc.vector.scalar_tensor_tensor(
                lam_t[:], p_t[:], -2.0, sv_t[:], op0=Op.mult, op1=Op.mult
            )
            nc.vector.tensor_add(lam_t[:], lam_t[:], q_t[:])
            den_t = statp.tile([128, NT], DT)
            nc.vector.tensor_scalar_add(den_t[:], tr_t[:], (K - 1) * 1e-6)
            dr_t = statp.tile([128, NT], DT)
            nc.vector.reciprocal(dr_t[:], den_t[:])
            curv_all = statp.tile([128, NT], DT)
            nc.vector.tensor_mul(curv_all[:], lam_t[:], dr_t[:])

            # ---- assemble flo rows ----
            # curv row: transpose to [8(t), 128(g)] then DMA-broadcast over k
            cps = ppsum.tile([8, 128], DT, tag="pp")
            nc.tensor.transpose(cps[:], curv_all[:], ident)
            ctv = statp.tile([8, 128], DT)
            nc.vector.tensor_copy(ctv[:], cps[:])
            curv_g = statp.tile([1, G], DT)
            nc.scalar.dma_start(
                curv_g.rearrange("o (t g) -> o t g", g=128), ctv[:]
            )

            # mean rows -> m3 [3, 1024] (d-partition, g-free)


            # lap rows: |xdkT - mean| with mean replicated over k by PE matmul
            lapT = statp.tile([96, G], DT)
            for half in range(2):
                sl = slice(half * 512, (half + 1) * 512)
                mrep = ppsum.tile([96, 512], DT, tag="pp")
                nc.tensor.matmul(
                    mrep[:],
                    rmat[:],
                    m3[:, sl],
                    start=True,
                    stop=True,
                )
                nc.vector.tensor_sub(lapT[:, sl], xdkT[:, sl].bitcast(DT), mrep[:])
            nc.scalar.activation(lapT[:], lapT[:], Act.Abs)

            # g42 row: gaussian of the d=2 plane of xdkT
            g42f = statp.tile([96, G], DT)
            g42_inst = nc.vector.scalar_tensor_tensor(
                g42f[:],
                xdkT[:].bitcast(DT),
                c242_t[:],
                xdkT[:].bitcast(DT),
                op0=Op.add,
                op1=Op.mult,
            )
            add_dep_helper(
                g42_inst.ins,
                sin_inst.ins,
                sync=True,
                reason="ACT table order: trig before exp",
            )
            nc.scalar.activation(g42f[:], g42f[:], Act.Exp, bias=ce42_t[:], scale=-2.0)

            # ones row source (folds projection bias into the matmul)
            HK = K // 4
            ones_t = gwp.tile([128, HK * G // 128], DT, tag="ones", bufs=1)
            nc.vector.memset(ones_t[:], 1.0)

            ppsum.release()
            xbp = tc.alloc_tile_pool(name="xbpsum", bufs=2, space="PSUM")
            outp = tc.alloc_tile_pool(name="outpsum", bufs=3, space="PSUM")

            # ---- main loop: four phases of 8 k each (flo quarter, double-buffered)
            # fusedT low rows [g42rbf, curv, lap0..2, ones]; cols (k - k0, g)
            for phase in range(4):
                k0 = phase * HK
                flo = flop.tile([6, HK * G], DTR, tag="flo", bufs=2)
                nc.scalar.dma_start(
                    flo[0:1, :].rearrange("o (k g) -> o k g", g=G),
                    g42f[2 * K + k0 : 2 * K + k0 + HK, :].bitcast(DTR),
                )
                nc.scalar.dma_start(
                    flo[1:2, :].rearrange("o (k g) -> o k g", g=G),
                    curv_g.bitcast(DTR).unsqueeze(1).broadcast_to([1, HK, G]),
                )
                for d in range(D):
                    nc.scalar.dma_start(
                        flo[2 + d : 3 + d, :].rearrange("o (k g) -> o k g", g=G),
                        lapT[d * K + k0 : d * K + k0 + HK, :].bitcast(DTR),
                    )
                nc.scalar.dma_start(
                    flo[5:6, :].rearrange("o (a b) -> o a b", b=HK * G // 128),
                    ones_t.bitcast(DTR),
                )
                for k in range(k0, k0 + HK):
                    for half in range(2):
                        csl = slice(half * 512, (half + 1) * 512)
                        xb = xbp.tile([128, 512], DT, tag="xb")
                        nc.tensor.matmul(
                            xb[:],
                            ebig[:, k * 128 : (k + 1) * 128],
                            xdkT[:, csl],
                            start=True,
                            stop=True,
                        )
                        # rbf = Exp(-2 * (x - c)^2), c per rbf row
                        t2 = mainp.tile([128, 512], DT, tag="t2")
                        nc.scalar.activation(t2[:], xb[:], Act.Square, bias=negc)
                        fhi = mainp.tile([128, 512], DTR, tag="fhi")
                        nc.scalar.activation(fhi[:], t2[:], Act.Exp, scale=-2.0)
                        so = mainp.tile([128, 4 * OUT], DT, tag="so", bufs=6)
                        for pair in range(2):
                            ps = outp.tile([128, 1024], DT, tag="ps")
                            for c in range(2):
                                j = pair * 2 + c
                                nc.tensor.matmul(
                                    ps[:, c * 512 : c * 512 + OUT],
                                    fhi[:, j * 128 : (j + 1) * 128],
                                    whi,
                                    start=True,
                                    stop=False,
                                )
                                lo = (k - k0) * G + half * 512 + j * 128
                                nc.tensor.matmul(
                                    ps[:, c * 512 : c * 512 + OUT],
                                    flo[0:6, lo : lo + 128],
                                    wlo,
                                    start=False,
                                    stop=True,
                                )
                            ps_v = ps.rearrange("p (c x) -> p c x", x=512)[:, :, 0:OUT]
                            so_v = so.rearrange("p (c x) -> p c x", x=OUT)[
                                :, pair * 2 : pair * 2 + 2, :
                            ]
                            cnt = (k * 2 + half) * 2 + pair
                            if cnt % 2 == 1:
                                nc.scalar.copy(so_v, ps_v)
                            else:
                                nc.vector.tensor_copy(so_v, ps_v)
                        # rows p = g*32 + k, g = half*512 .. +512
                        out_ap = (
                            out_d.rearrange("(g k) x -> k g x", k=K)[
                                k : k + 1, half * 512 : (half + 1) * 512, :
                            ]
                            .squeeze(0)
                            .rearrange("(c g) x -> g c x", c=4)
                        )
                        nc.sync.dma_start(
                            out_ap, so.rearrange("p (c x) -> p c x", x=OUT)
                        )
            outp.release()
            xbp.release()

    nc.compile()
    return nc


def _get_program():
    if "nc" not in _prog_cache:
        _prog_cache["nc"] = _build_program()
    return _prog_cache["nc"]


def kernel(xyz, neighbor_xyz, projection_weight, projection_bias):
    from concourse.bass_utils import run_bass_kernel_spmd

    nc = _get_program()

    w = np.ascontiguousarray(projection_weight, dtype=np.float32)
    bias = np.ascontiguousarray(projection_bias, dtype=np.float32)
    whi = np.ascontiguousarray(w[:128])
    wlo6 = np.concatenate([w[128:TOTAL], bias[None, :]], axis=0).astype(np.float32)


    ebig = np.zeros((96, K * 128), dtype=np.float32)
    for k in range(K):
        for r in range(128):
            ebig[(r // F) * K + k, k * 128 + r] = 1.0
    rmat = np.zeros((D, 96), dtype=np.float32)
    for d in range(D):
        rmat[d, d * K : (d + 1) * K] = 1.0
    ident = np.eye(128, dtype=np.float32)
    c = np.linspace(-1.0, 1.0, F + 2, dtype=np.float32)[1:-1]
    cr = c[np.arange(128) % F]
    negc = (-cr).reshape(128, 1).astype(np.float32)
    blob1 = np.zeros((128, 128 + OUT + 1 + 96), dtype=np.float32)
    blob1[:, 0:128] = ident
    blob1[0:6, 128 : 128 + OUT] = wlo6
    blob1[:, 128 + OUT : 128 + OUT + 1] = negc
    blob1[0:D, 128 + OUT + 1 :] = rmat
    blob2 = np.zeros((128, OUT + K * 128), dtype=np.float32)
    blob2[:, 0:OUT] = whi
    blob2[0:96, OUT:] = ebig

    xyz = np.ascontiguousarray(xyz, dtype=np.float32)
    nbr = np.ascontiguousarray(neighbor_xyz, dtype=np.float32)
    in_maps = []
    for core in range(B):
        in_maps.append(
            {
                "xyz": xyz[core].reshape(P, D),
                "nbr": nbr[core].reshape(P, D),
                "blob1": blob1,
                "blob2": blob2,
            }
        )
    _prog_cache["last_in_maps"] = in_maps
    globals()["_last_in_maps"] = in_maps
    res = run_bass_kernel_spmd(nc, in_maps, list(range(B)))
    out = np.stack(
        [res.results[i]["out"].reshape(S, K, OUT) for i in range(B)], axis=0
    )
    return out

